# revision 1
# baseline (speedup 1.0000x reference)
"""Spikformer block (Q/K/V linear+BN+{LIF,ReLU,ternary} -> headwise linear attention
-> O linear+BN+LIF) on 8 TRN2 NeuronCores, data-parallel over batch.

Self-contained: hardcodes shapes; builds one SPMD Bass program; shards batch
across 8 cores; gathers/transposes on host.

Key algebra:
  - attention has no softmax -> (q k^T) v reassociated as q (k^T v); per-head
    k^T v is 64x64, ~8x fewer MACs and no 512x512 attn matrix to evict.
  - all BatchNorms are eval-mode affine: folded into per-channel scale/bias on
    host; applied during PSUM eviction (per-partition APs) for the [e,t]-layout
    paths, or folded into weights + a bias matmul for the [t,e]-layout paths.
"""
import sys
for p in ("/opt/trn_rl_repo",):
    if p not in sys.path:
        sys.path.insert(0, p)

import numpy as np
import ml_dtypes

import concourse.bass as bass
import concourse.bacc as bacc
import concourse.mybir as mybir
import concourse.tile as tile
from concourse.bass_utils import run_bass_kernel_spmd

B, T, L, D, H = 8, 4, 512, 512, 8
HD = D // H
NT = T * L            # 2048 tokens per core
P = 128
EC = D // P           # 4 e-chunks
DC = D // P           # 4 d-chunks
TC = NT // P          # 16 token chunks (choice-2 layout)
EPS = 1e-5
F32 = mybir.dt.float32
BF16 = mybir.dt.bfloat16
F = mybir.ActivationFunctionType
ALU = mybir.AluOpType

LIN_MODE = "bf16x3"   # "fp32" (native, 4cyc/row) or "bf16x3" (split, 3cyc/row)

_PROGRAM_CACHE = {}
_last_in_maps = None


def _build_program():
    key = LIN_MODE
    if key in _PROGRAM_CACHE:
        return _PROGRAM_CACHE[key]

    nc = bacc.Bacc("TRN2", target_bir_lowering=False, debug=False, num_devices=8)

    dram = {}
    if LIN_MODE == "fp32":
        dram["xT"] = nc.dram_tensor("xT", [D, NT], F32, kind="ExternalInput")
        for w in ("wq", "wk", "wv", "wo"):
            dram[w] = nc.dram_tensor(w, [D, D], F32, kind="ExternalInput")
    else:
        for t_ in ("xTh", "xTl"):
            dram[t_] = nc.dram_tensor(t_, [D, NT], BF16, kind="ExternalInput")
        for w in ("wq", "wk", "wv", "wo"):
            for s in ("h", "l"):
                dram[w + s] = nc.dram_tensor(w + s, [D, D], BF16, kind="ExternalInput")
    for v_ in ("qs", "qb", "os_", "ob"):
        dram[v_] = nc.dram_tensor(v_, [D, 1], F32, kind="ExternalInput")
    dram["kbb"] = nc.dram_tensor("kbb", [P, D], F32, kind="ExternalInput")
    dram["vthr1"] = nc.dram_tensor("vthr1", [P, D], F32, kind="ExternalInput")
    dram["vthr2"] = nc.dram_tensor("vthr2", [P, D], F32, kind="ExternalInput")
    out_d = nc.dram_tensor("out", [D, NT], BF16, kind="ExternalOutput")

    with tile.TileContext(nc) as tc_:
        with tc_.tile_pool(name="sb", bufs=1) as sb, \
             tc_.tile_pool(name="sc", bufs=3) as sc, \
             tc_.tile_pool(name="sp8", bufs=6) as sp8, \
             tc_.tile_pool(name="ps", bufs=5, space="PSUM") as ps, \
             tc_.tile_pool(name="pk", bufs=3, space="PSUM") as pk:

            # ---------- persistent SBUF tiles ----------
            if LIN_MODE == "fp32":
                xT = [sb.tile([P, NT], F32, tag="xa", bufs=DC, name=f"xT{i}") for i in range(DC)]
                wts = {w: [sb.tile([P, D], F32, tag=w, bufs=DC, name=f"{w}{i}") for i in range(DC)]
                       for w in ("wq", "wk", "wv", "wo")}
            else:
                xTh = [sb.tile([P, NT], BF16, tag="xa", bufs=2 * DC, name=f"xTh{i}") for i in range(DC)]
                xTl = [sb.tile([P, NT], BF16, tag="xa", bufs=2 * DC, name=f"xTl{i}") for i in range(DC)]
                wts = {w + s: [sb.tile([P, D], BF16, tag=w + s, bufs=DC, name=f"{w}{s}{i}") for i in range(DC)]
                       for w in ("wq", "wk", "wv", "wo") for s in ("h", "l")}
            qT = [sb.tile([P, NT], BF16, tag="qT", bufs=EC, name=f"qT{i}") for i in range(EC)]       # q spikes [e,t]
            k_hi = [sb.tile([P, D], BF16, tag="khi", bufs=TC, name=f"khi{i}") for i in range(TC)]
            k_lo = [sb.tile([P, D], BF16, tag="klo", bufs=TC, name=f"klo{i}") for i in range(TC)]   # k [t,e]
            v_nat = [sb.tile([P, D], BF16, tag="vnat", bufs=TC, name=f"vnat{i}") for i in range(TC)]   # v [t,e]
            # attention output [e, t] as bf16 hi/lo pairs (exact enough for o-linear)
            if LIN_MODE == "fp32":
                ao = [sb.tile([P, NT], F32, tag="xa", bufs=DC, name=f"ao{i}") for i in range(DC)]
            else:
                aoh = [sb.tile([P, NT], BF16, tag="xa", bufs=2 * DC, name=f"aoh{i}") for i in range(DC)]
                aol = [sb.tile([P, NT], BF16, tag="xa", bufs=2 * DC, name=f"aol{i}") for i in range(DC)]
            memq = [sb.tile([P, L], F32, tag="memq", bufs=EC, name=f"memq{i}") for i in range(EC)]
            memo = [sb.tile([P, L], F32, tag="memo", bufs=EC, name=f"memo{i}") for i in range(EC)]
            consts = {v_: [sb.tile([P, 1], F32, tag="cst", bufs=4 * EC, name=f"c_{v_}{i}") for i in range(EC)]
                      for v_ in ("qs", "qb", "os_", "ob")}
            kbb = sb.tile([P, D], F32, tag="kbb")
            vthr1 = sb.tile([P, D], F32, tag="vthr1")
            vthr2 = sb.tile([P, D], F32, tag="vthr2")
            cneg1 = sb.tile([P, 1], F32, tag="cneg1")
            nc.gpsimd.memset(cneg1[:], -1.0)
            zrow = sb.tile([1, P], BF16, tag="zrow")
            nc.gpsimd.memset(zrow[:], 0.0)

            # ---------- loads ----------
            if LIN_MODE == "fp32":
                for dc in range(DC):
                    nc.sync.dma_start(xT[dc][:], dram["xT"][dc * P:(dc + 1) * P, :])
                for w in ("wq", "wk", "wv", "wo"):
                    for dc in range(DC):
                        nc.sync.dma_start(wts[w][dc][:], dram[w][dc * P:(dc + 1) * P, :])
            else:
                for w in ("wqh", "wql"):
                    for dc in range(DC):
                        nc.sync.dma_start(wts[w][dc][:], dram[w][dc * P:(dc + 1) * P, :])
                for v_ in consts:
                    for i in range(EC):
                        nc.sync.dma_start(consts[v_][i][:], dram[v_][i * P:(i + 1) * P, :])
                nc.sync.dma_start(kbb[:], dram["kbb"][:])
                nc.sync.dma_start(vthr1[:], dram["vthr1"][:])
                nc.sync.dma_start(vthr2[:], dram["vthr2"][:])
                for q4 in range(T):
                    cs = slice(q4 * L, (q4 + 1) * L)
                    for dc in range(DC):
                        nc.scalar.dma_start(xTh[dc][:, cs], dram["xTh"][dc * P:(dc + 1) * P, cs])
                        nc.scalar.dma_start(xTl[dc][:, cs], dram["xTl"][dc * P:(dc + 1) * P, cs])
                for w in ("wkh", "wkl", "wvh", "wvl", "woh", "wol"):
                    for dc in range(DC):
                        nc.sync.dma_start(wts[w][dc][:], dram[w][dc * P:(dc + 1) * P, :])

            def lin_mms(psum, w, lhs_xt=False, tc_=None, ec=None, ti=None, close=False):
                """Emit the matmul group for one linear output tile.
                close=True marks stop on the final matmul (no trailing bias mm)."""
                if LIN_MODE == "fp32":
                    for dc in range(DC):
                        if lhs_xt:   # choice-2: lhsT = xT chunk cols, rhs = weight
                            lhsT = xT[dc][:, tc_ * P:(tc_ + 1) * P]
                            rhs = wts[w][dc][:]
                        else:        # choice-1: lhsT = weight cols, rhs = xT cols
                            lhsT = wts[w][dc][:, ec * P:(ec + 1) * P]
                            rhs = xT[dc][:, ti * L:(ti + 1) * L]
                        nc.tensor.matmul(psum[:], lhsT, rhs,
                                         start=(dc == 0),
                                         stop=(close and dc == DC - 1))
                else:
                    i = 0
                    n = 3 * DC
                    for dc in range(DC):
                        for (xs, wsfx) in ((xTh, "h"), (xTh, "l"), (xTl, "h")):
                            i += 1
                            if lhs_xt:
                                lhsT = xs[dc][:, tc_ * P:(tc_ + 1) * P]
                                rhs = wts[w + wsfx][dc][:]
                            else:
                                lhsT = wts[w + wsfx][dc][:, ec * P:(ec + 1) * P]
                                rhs = xs[dc][:, ti * L:(ti + 1) * L]
                            nc.tensor.matmul(psum[:], lhsT, rhs,
                                             start=(i == 1),
                                             stop=(close and i == n))

            # ---------- phase 1: Q linear + LIF ----------
            for ti in range(T):
                for ec in range(EC):
                    pq = ps.tile([P, L], F32, tag="mm512", name=f"pq{ti}_{ec}")
                    lin_mms(pq, "wq", ec=ec, ti=ti, close=True)
                    u = sc.tile([P, L], F32, tag="u", name=f"u{ti}_{ec}")
                    if ti == 0:
                        nc.vector.tensor_scalar(memq[ec][:], pq[:], consts["qs"][ec][:],
                                                consts["qb"][ec][:], ALU.mult, ALU.add)
                    else:
                        nc.vector.tensor_scalar(u[:], pq[:], consts["qs"][ec][:],
                                                consts["qb"][ec][:], ALU.mult, ALU.add)
                        nc.vector.scalar_tensor_tensor(memq[ec][:], memq[ec][:], 0.5, u[:],
                                                       ALU.mult, ALU.add)
                        nc.gpsimd.tensor_sub(memq[ec][:], memq[ec][:],
                                             qT[ec][:, (ti - 1) * L:ti * L])
                    nc.vector.tensor_scalar(qT[ec][:, ti * L:(ti + 1) * L],
                                            memq[ec][:], 1.0, None, ALU.is_ge)

            # ---------- phase 1b: K and V linears ----------
            for tc2 in range(TC):
                pkv = ps.tile([P, D], F32, tag="mm512", name=f"pk{tc2}")
                lin_mms(pkv, "wk", lhs_xt=True, tc_=tc2, close=True)
                kf = sc.tile([P, D], F32, tag="kf", name=f"kf{tc2}")
                nc.vector.scalar_tensor_tensor(kf[:], pkv[:], 0.0, kbb[:],
                                               ALU.add, ALU.add)
                nc.scalar.activation(k_hi[tc2][:], kf[:], F.Relu)
                kf2 = sc.tile([P, D], F32, tag="kf2", name=f"kf2_{tc2}")
                nc.scalar.activation(kf2[:], kf[:], F.Relu)
                nc.vector.tensor_tensor(k_lo[tc2][:], kf2[:], k_hi[tc2][:], ALU.subtract)

                pvv = ps.tile([P, D], F32, tag="mm512", name=f"pv{tc2}")
                lin_mms(pvv, "wv", lhs_xt=True, tc_=tc2, close=True)
                t1 = sc.tile([P, D], BF16, tag="t1", name=f"t1_{tc2}")
                t2 = sc.tile([P, D], BF16, tag="t2", name=f"t2_{tc2}")
                nc.vector.tensor_tensor(t1[:], pvv[:], vthr1[:], ALU.is_ge)
                nc.vector.tensor_tensor(t2[:], pvv[:], vthr2[:], ALU.is_le)
                nc.gpsimd.tensor_sub(v_nat[tc2][:], t1[:], t2[:])

            # ---------- phase 2: attention  out^T[e,l] = kv^T q^T ----------
            for ti in range(T):
                kvsplits = []
                for c in range(EC):          # head pair (2c, 2c+1) == e-chunk c
                    pkv64 = pk.tile([P, 2 * HD], F32, tag="kv64", name=f"pkv{ti}_{c}")
                    # diagonal head blocks: per-head matmuls into disjoint
                    # partition/col strips; off-diagonal blocks zeroed by K=1
                    # zero-matmuls so one K=128 q@kv matmul serves both heads.
                    for hh in range(2):
                        h = 2 * c + hh
                        off = hh * HD
                        hs = slice(h * HD, (h + 1) * HD)
                        i = 0
                        for mc in range(4):
                            tc2 = ti * 4 + mc
                            for kk in (k_hi, k_lo):
                                i += 1
                                nc.tensor.matmul(
                                    pkv64[off:off + HD, off:off + HD],
                                    kk[tc2][:, hs], v_nat[tc2][:, hs],
                                    start=(i == 1), stop=(i == 8),
                                    tile_position=(0, off))
                        ofo = HD - off
                        nc.tensor.matmul(pkv64[off:off + HD, ofo:ofo + HD],
                                         zrow[:, 0:HD], zrow[:, 0:HD],
                                         start=True, stop=True,
                                         tile_position=(0, off))
                    kv1 = sc.tile([P, 2 * HD], BF16, tag="kv1", bufs=5, name=f"kv1_{ti}_{c}")
                    kv2 = sc.tile([P, 2 * HD], BF16, tag="kv2", bufs=5, name=f"kv2_{ti}_{c}")
                    nc.scalar.copy(kv1[:], pkv64[:])
                    nc.vector.tensor_tensor(kv2[:], pkv64[:], kv1[:], ALU.subtract)
                    kvsplits.append((kv1, kv2))
                for c in range(EC):
                    kv1, kv2 = kvsplits[c]
                    pso = ps.tile([P, L], F32, tag="mm512", name=f"pso{ti}_{c}")
                    for j, kvt in enumerate((kv1, kv2)):
                        nc.tensor.matmul(pso[:], kvt[:],
                                         qT[c][:, ti * L:(ti + 1) * L],
                                         start=(j == 0), stop=(j == 1))
                    if LIN_MODE == "fp32":
                        nc.scalar.copy(ao[c][:, ti * L:(ti + 1) * L], pso[:])
                    else:
                        nc.scalar.copy(aoh[c][:, ti * L:(ti + 1) * L], pso[:])
                        nc.vector.tensor_tensor(aol[c][:, ti * L:(ti + 1) * L],
                                                pso[:], aoh[c][:, ti * L:(ti + 1) * L],
                                                ALU.subtract)

            # ---------- phase 3: O linear + BN + LIF ----------
            prev_spk = [None] * EC
            for ti in range(T):
                for ec in range(EC):
                    po = ps.tile([P, L], F32, tag="mm512")
                    if LIN_MODE == "fp32":
                        for dc in range(DC):
                            nc.tensor.matmul(po[:], wts["wo"][dc][:, ec * P:(ec + 1) * P],
                                             ao[dc][:, ti * L:(ti + 1) * L],
                                             start=(dc == 0), stop=(dc == DC - 1))
                    else:
                        first = True
                        n3 = 3 * DC
                        i = 0
                        for (asrc, wsfx) in ((aoh, "h"), (aoh, "l"), (aol, "h")):
                            for dc in range(DC):
                                i += 1
                                nc.tensor.matmul(po[:], wts["wo" + wsfx][dc][:, ec * P:(ec + 1) * P],
                                                 asrc[dc][:, ti * L:(ti + 1) * L],
                                                 start=first, stop=(i == n3))
                                first = False
                    u = sc.tile([P, L], F32, tag="u")
                    if ti == 0:
                        nc.vector.tensor_scalar(memo[ec][:], po[:], consts["os_"][ec][:],
                                                consts["ob"][ec][:], ALU.mult, ALU.add)
                    else:
                        nc.vector.tensor_scalar(u[:], po[:], consts["os_"][ec][:],
                                                consts["ob"][ec][:], ALU.mult, ALU.add)
                        nc.vector.scalar_tensor_tensor(memo[ec][:], memo[ec][:], 0.5, u[:],
                                                       ALU.mult, ALU.add)
                        nc.gpsimd.tensor_sub(memo[ec][:], memo[ec][:], prev_spk[ec][:])
                    spk = sp8.tile([P, L], BF16, tag="ospk")
                    nc.vector.tensor_scalar(spk[:], memo[ec][:], 1.0, None, ALU.is_ge)
                    prev_spk[ec] = spk
                    nc.sync.dma_start(out_d[ec * P:(ec + 1) * P, ti * L:(ti + 1) * L], spk[:])

    nc.compile()
    _PROGRAM_CACHE[key] = nc
    return nc


def _split_bf16(a):
    hi = a.astype(ml_dtypes.bfloat16)
    lo = (a - hi.astype(np.float32)).astype(ml_dtypes.bfloat16)
    return hi, lo


def _split3_bf16(a):
    h1 = a.astype(ml_dtypes.bfloat16)
    r = a - h1.astype(np.float32)
    h2 = r.astype(ml_dtypes.bfloat16)
    h3 = (r - h2.astype(np.float32)).astype(ml_dtypes.bfloat16)
    return np.stack([h1, h2, h3])


def kernel(**inputs):
    nc = _build_program()

    f64 = np.float64
    x = np.asarray(inputs["x"], np.float32)

    def bn_fold(g, b_, rm, rv):
        s = (g.astype(f64) / np.sqrt(rv.astype(f64) + EPS))
        bias = b_.astype(f64) - rm.astype(f64) * s
        return s, bias

    sq, bq = bn_fold(inputs["q_g"], inputs["q_b"], inputs["q_rm"], inputs["q_rv"])
    sk, bk = bn_fold(inputs["k_g"], inputs["k_b"], inputs["k_rm"], inputs["k_rv"])
    sv, bv = bn_fold(inputs["v_g"], inputs["v_b"], inputs["v_rm"], inputs["v_rv"])
    so, bo = bn_fold(inputs["o_g"], inputs["o_b"], inputs["o_rm"], inputs["o_rv"])
    C = HD ** -0.5
    # o path: out = bn(lin + o_bias) -> bias' = (o_bias - rm)*s + b
    bo = bo + inputs["o_bias"].astype(f64) * so

    wq = inputs["q_w"].astype(f64)
    wk = inputs["k_w"].astype(f64) * (C * sk)[:, None]
    wv = inputs["v_w"].astype(f64) * sv[:, None]
    wo = inputs["o_w"].astype(f64)
    kb_fold = (C * bk).astype(np.float32)
    vb_fold = bv.astype(np.float32)

    shared = {
        "qs": sq.astype(np.float32).reshape(D, 1),
        "qb": bq.astype(np.float32).reshape(D, 1),
        "os_": so.astype(np.float32).reshape(D, 1),
        "ob": bo.astype(np.float32).reshape(D, 1),
        "kbb": np.ascontiguousarray(np.broadcast_to(kb_fold[None, :], (P, D))),
        "vthr1": np.ascontiguousarray(np.broadcast_to((np.float32(1.0) - vb_fold)[None, :], (P, D))),
        "vthr2": np.ascontiguousarray(np.broadcast_to((np.float32(-1.0) - vb_fold)[None, :], (P, D))),
    }
    if LIN_MODE == "fp32":
        shared["wq"] = np.ascontiguousarray(wq.astype(np.float32).T)
        shared["wk"] = np.ascontiguousarray(wk.astype(np.float32).T)
        shared["wv"] = np.ascontiguousarray(wv.astype(np.float32).T)
        shared["wo"] = np.ascontiguousarray(wo.astype(np.float32).T)
    else:
        for name, w in (("wq", wq), ("wk", wk), ("wv", wv), ("wo", wo)):
            hi, lo = _split_bf16(np.ascontiguousarray(w.astype(np.float32).T))
            shared[name + "h"] = hi
            shared[name + "l"] = lo

    in_maps = []
    for b in range(B):
        xT = np.ascontiguousarray(x[b].reshape(NT, D).T)   # (D, NT)
        m = dict(shared)
        if LIN_MODE == "fp32":
            m["xT"] = xT
        else:
            hi, lo = _split_bf16(xT)
            m["xTh"] = hi
            m["xTl"] = lo
        in_maps.append(m)

    global _last_in_maps
    _last_in_maps = in_maps
    res = run_bass_kernel_spmd(nc, in_maps, core_ids=list(range(B)))
    outs = []
    for b in range(B):
        oT = res.results[b]["out"]                    # (D, NT)
        outs.append(oT.reshape(D, T, L).transpose(1, 2, 0))
    return np.stack(outs).astype(np.float32)


if __name__ == "__main__":
    import importlib.util
    spec = importlib.util.spec_from_file_location("reference", "/root/problem/reference.py")
    ref = importlib.util.module_from_spec(spec)
    spec.loader.exec_module(ref)
    inp = {k: np.asarray(v) for k, v in ref.setup_inputs().items()}
    exp = np.asarray(ref.reference(**inp))
    act = kernel(**inp)
    rel = np.linalg.norm(act - exp) / np.linalg.norm(exp)
    print("flips:", int(np.sum(act != exp)), "/", exp.size)
    print("Relative error:", rel)



# revision 4
# speedup vs baseline: 1.3087x; 1.3087x over previous
"""Spikformer block (Q/K/V linear+BN+{LIF,ReLU,ternary} -> headwise linear attention
-> O linear+BN+LIF) on 8 TRN2 NeuronCores, data-parallel over batch.

Self-contained: hardcodes shapes; builds one SPMD Bass program; shards batch
across 8 cores; gathers/transposes on host.

Key algebra / precision plan:
  - attention has no softmax -> (q k^T) v reassociated as q (k^T v); per-head
    k^T v is 64x64, ~8x fewer MACs and no 512x512 attn matrix to evict.
  - all BatchNorms are eval-mode affine: folded into per-channel scale/bias on
    host.
  - matmul dtypes tuned per-path to the LIF/ternary flip sensitivity
    (measured on the reference inputs):
      * fp32r streams at 1 cyc/row (free dim >= 256) and rounds operands to
        12-bit significands (RNE, verified on silicon).
      * Q linear: 2 passes  (fp32r hi + bf16 residual-of-x)
      * K linear: 1 pass    (fp32r; errors average out in the k^T v sum)
      * V linear: 3 passes  (fp32r hi + bf16 x-residual + bf16 w-residual);
        ternary thresholding is the most flip-sensitive spot.
      * O linear: 1 pass    (fp32r)
      * attention: fp16 single-pass (q binary and v ternary are exact in
        fp16; k/kv quantization adds ~1e-4 relative, tolerable).
  - k^T v computed as one 128-wide matmul per token chunk covering 2 heads;
    the cross-head off-diagonal blocks are garbage but never read: only the
    two diagonal 64x64 blocks are copied into pre-zeroed kv tiles.
"""
import sys
for p in ("/opt/trn_rl_repo",):
    if p not in sys.path:
        sys.path.insert(0, p)

import numpy as np
import ml_dtypes

import concourse.bass as bass
import concourse.bacc as bacc
import concourse.mybir as mybir
import concourse.tile as tile
from concourse.bass_utils import run_bass_kernel_spmd

B, T, L, D, H = 8, 4, 512, 512, 8
HD = D // H
NT = T * L            # 2048 tokens per core
P = 128
EC = D // P           # 4 e-chunks
DC = D // P           # 4 d-chunks
TC = NT // P          # 16 token chunks
EPS = 1e-5
F32 = mybir.dt.float32
F32R = mybir.dt.float32r
BF16 = mybir.dt.bfloat16
FP16 = mybir.dt.float16
F = mybir.ActivationFunctionType
ALU = mybir.AluOpType

_PROGRAM_CACHE = {}
_last_in_maps = None


def _r(ap):
    return ap.bitcast(F32R)


def _build_program():
    if "nc" in _PROGRAM_CACHE:
        return _PROGRAM_CACHE["nc"]

    nc = bacc.Bacc("TRN2", target_bir_lowering=False, debug=False, num_devices=8)

    dram = {}
    dram["xT"] = nc.dram_tensor("xT", [D, NT], F32R, kind="ExternalInput")
    dram["xlT"] = nc.dram_tensor("xlT", [D, NT], BF16, kind="ExternalInput")
    dram["xbT"] = nc.dram_tensor("xbT", [D, NT], BF16, kind="ExternalInput")
    for w in ("wq", "wk", "wv", "wo"):
        dram[w] = nc.dram_tensor(w, [D, D], F32R, kind="ExternalInput")
    for w in ("wq8", "wv8", "wvl8"):
        dram[w] = nc.dram_tensor(w, [D, D], BF16, kind="ExternalInput")
    for v_ in ("qs", "qb", "os_", "ob"):
        dram[v_] = nc.dram_tensor(v_, [D, 1], F32, kind="ExternalInput")
    dram["kbb"] = nc.dram_tensor("kbb", [P, D], F32, kind="ExternalInput")
    dram["vthr1"] = nc.dram_tensor("vthr1", [P, D], F32, kind="ExternalInput")
    dram["vthr2"] = nc.dram_tensor("vthr2", [P, D], F32, kind="ExternalInput")
    out_d = nc.dram_tensor("out", [D, NT], BF16, kind="ExternalOutput")

    with tile.TileContext(nc) as tc_:
        with tc_.tile_pool(name="sb", bufs=1) as sb, \
             tc_.tile_pool(name="sc", bufs=3) as sc, \
             tc_.tile_pool(name="sp8", bufs=6) as sp8, \
             tc_.tile_pool(name="ps", bufs=5, space="PSUM") as ps, \
             tc_.tile_pool(name="pk", bufs=2, space="PSUM") as pk:

            # ---------- persistent SBUF tiles ----------
            xT = [sb.tile([P, NT], F32R, tag="xa", bufs=DC, name=f"xT{i}") for i in range(DC)]
            xlT = [sb.tile([P, NT], BF16, tag="xl", bufs=DC, name=f"xlT{i}") for i in range(DC)]
            xbT = [sb.tile([P, NT], BF16, tag="xb", bufs=DC, name=f"xbT{i}") for i in range(DC)]
            wts = {}
            for w in ("wq", "wk", "wv", "wo"):
                wts[w] = [sb.tile([P, D], F32R, tag=w, bufs=DC, name=f"{w}{i}") for i in range(DC)]
            for w in ("wq8", "wv8", "wvl8"):
                wts[w] = [sb.tile([P, D], BF16, tag=w, bufs=DC, name=f"{w}{i}") for i in range(DC)]
            qT16 = [sb.tile([P, NT], FP16, tag="qT", bufs=EC, name=f"qT{i}") for i in range(EC)]
            k16 = [sb.tile([P, D], FP16, tag="k16", bufs=TC, name=f"k16_{i}") for i in range(TC)]
            v16 = [sb.tile([P, D], FP16, tag="v16", bufs=TC, name=f"v16_{i}") for i in range(TC)]
            kv16 = [sb.tile([P, P], FP16, tag="kv16", bufs=TC, name=f"kv16_{i}") for i in range(TC)]
            # attention output [e, t] fp32; reuses the xT buffers (tag "xa"),
            # safe because all xT reads precede phase 2.
            ao = [sb.tile([P, NT], F32R, tag="xa", bufs=DC, name=f"ao{i}") for i in range(DC)]
            memq = [sb.tile([P, L], F32, tag="memq", bufs=EC, name=f"memq{i}") for i in range(EC)]
            memo = [sb.tile([P, L], F32, tag="memo", bufs=EC, name=f"memo{i}") for i in range(EC)]
            consts = {v_: [sb.tile([P, 1], F32, tag="cst", bufs=4 * EC, name=f"c_{v_}{i}") for i in range(EC)]
                      for v_ in ("qs", "qb", "os_", "ob")}
            kbb = sb.tile([P, D], F32, tag="kbb")
            vthr1 = sb.tile([P, D], F32, tag="vthr1")
            vthr2 = sb.tile([P, D], F32, tag="vthr2")

            # kv tiles: zero once; only diagonal 64x64 blocks are ever written
            for i in range(TC):
                nc.gpsimd.memset(kv16[i][:], 0.0)

            # ---------- loads ----------
            # sync queue: weights/consts in phase order
            for dc in range(DC):
                nc.sync.dma_start(wts["wq"][dc][:], dram["wq"][dc * P:(dc + 1) * P, :])
            for dc in range(DC):
                nc.sync.dma_start(wts["wq8"][dc][:], dram["wq8"][dc * P:(dc + 1) * P, :])
            for v_ in ("qs", "qb"):
                for i in range(EC):
                    nc.sync.dma_start(consts[v_][i][:], dram[v_][i * P:(i + 1) * P, :])
            for w in ("wk", "wv", "wv8", "wvl8"):
                for dc in range(DC):
                    nc.sync.dma_start(wts[w][dc][:], dram[w][dc * P:(dc + 1) * P, :])
            nc.sync.dma_start(kbb[:], dram["kbb"][:])
            nc.sync.dma_start(vthr1[:], dram["vthr1"][:])
            nc.sync.dma_start(vthr2[:], dram["vthr2"][:])
            for dc in range(DC):
                nc.sync.dma_start(wts["wo"][dc][:], dram["wo"][dc * P:(dc + 1) * P, :])
            for v_ in ("os_", "ob"):
                for i in range(EC):
                    nc.sync.dma_start(consts[v_][i][:], dram[v_][i * P:(i + 1) * P, :])
            # scalar queue: x quarters in consumption order
            for q4 in range(T):
                cs = slice(q4 * L, (q4 + 1) * L)
                for dc in range(DC):
                    nc.scalar.dma_start(xT[dc][:, cs], dram["xT"][dc * P:(dc + 1) * P, cs])
                for dc in range(DC):
                    nc.scalar.dma_start(xlT[dc][:, cs], dram["xlT"][dc * P:(dc + 1) * P, cs])
                for dc in range(DC):
                    nc.scalar.dma_start(xbT[dc][:, cs], dram["xbT"][dc * P:(dc + 1) * P, cs])

            # ---------- phase 1: Q linear (2-pass) + BN + LIF ----------
            for ti in range(T):
                xs = slice(ti * L, (ti + 1) * L)
                for ec in range(EC):
                    es = slice(ec * P, (ec + 1) * P)
                    pq = ps.tile([P, L], F32, tag="mm512", name=f"pq{ti}_{ec}")
                    for dc in range(DC):
                        nc.tensor.matmul(pq[:], wts["wq"][dc][:, es], xT[dc][:, xs],
                                         start=(dc == 0), stop=False)
                    for dc in range(DC):
                        nc.tensor.matmul(pq[:], wts["wq8"][dc][:, es], xlT[dc][:, xs],
                                         start=False, stop=(dc == DC - 1))
                    if ti == 0:
                        nc.scalar.activation(memq[ec][:], pq[:], F.Identity,
                                             bias=consts["qb"][ec][:], scale=consts["qs"][ec][:])
                    else:
                        u = sc.tile([P, L], F32, tag="u", name=f"u{ti}_{ec}")
                        nc.scalar.activation(u[:], pq[:], F.Identity,
                                             bias=consts["qb"][ec][:], scale=consts["qs"][ec][:])
                        nc.vector.scalar_tensor_tensor(memq[ec][:], memq[ec][:], 0.5, u[:],
                                                       ALU.mult, ALU.add)
                        nc.gpsimd.tensor_sub(memq[ec][:], memq[ec][:],
                                             qT16[ec][:, (ti - 1) * L:ti * L])
                    nc.vector.tensor_scalar(qT16[ec][:, xs], memq[ec][:], 1.0, None, ALU.is_ge)

            # ---------- phase 1b: K (1-pass) and V (3-pass) linears ----------
            for tc2 in range(TC):
                cs = slice(tc2 * P, (tc2 + 1) * P)
                pkv = ps.tile([P, D], F32, tag="mm512", name=f"pk{tc2}")
                for dc in range(DC):
                    nc.tensor.matmul(pkv[:], xT[dc][:, cs], wts["wk"][dc][:],
                                     start=(dc == 0), stop=(dc == DC - 1))
                kf = sc.tile([P, D], F32, tag="kf", name=f"kf{tc2}")
                nc.vector.tensor_tensor(kf[:], pkv[:], kbb[:], ALU.add)
                nc.scalar.activation(k16[tc2][:], kf[:], F.Relu)

                pvv = ps.tile([P, D], F32, tag="mm512", name=f"pv{tc2}")
                for dc in range(DC):
                    nc.tensor.matmul(pvv[:], xT[dc][:, cs], wts["wv"][dc][:],
                                     start=(dc == 0), stop=False)
                for dc in range(DC):
                    nc.tensor.matmul(pvv[:], xlT[dc][:, cs], wts["wv8"][dc][:],
                                     start=False, stop=False)
                for dc in range(DC):
                    nc.tensor.matmul(pvv[:], xbT[dc][:, cs], wts["wvl8"][dc][:],
                                     start=False, stop=(dc == DC - 1))
                t1 = sc.tile([P, D], FP16, tag="t1", name=f"t1_{tc2}")
                t2 = sc.tile([P, D], FP16, tag="t2", name=f"t2_{tc2}")
                nc.vector.tensor_tensor(t1[:], pvv[:], vthr1[:], ALU.is_ge)
                nc.vector.tensor_tensor(t2[:], pvv[:], vthr2[:], ALU.is_le)
                nc.gpsimd.tensor_sub(v16[tc2][:], t1[:], t2[:])

            # ---------- phase 2: attention  ao^T[e,l] = kv^T q^T ----------
            for ti in range(T):
                xs = slice(ti * L, (ti + 1) * L)
                for c in range(EC):          # head pair (2c, 2c+1) == e-chunk c
                    es = slice(c * P, (c + 1) * P)
                    pkv64 = pk.tile([P, P], F32, tag="kv64", name=f"pkv{ti}_{c}")
                    for mc in range(4):
                        tc2 = ti * 4 + mc
                        nc.tensor.matmul(pkv64[:], k16[tc2][:, es], v16[tc2][:, es],
                                         start=(mc == 0), stop=(mc == 3))
                    kvt = kv16[ti * 4 + c]
                    nc.scalar.copy(kvt[0:HD, 0:HD], pkv64[0:HD, 0:HD])
                    nc.scalar.copy(kvt[HD:P, HD:P], pkv64[HD:P, HD:P])
                    pso = ps.tile([P, L], F32, tag="mm512", name=f"pso{ti}_{c}")
                    nc.tensor.matmul(pso[:], kvt[:], qT16[c][:, xs],
                                     start=True, stop=True)
                    nc.scalar.copy(ao[c][:, xs], pso[:])

            # ---------- phase 3: O linear (1-pass) + BN + LIF ----------
            prev_spk = [None] * EC
            for ti in range(T):
                xs = slice(ti * L, (ti + 1) * L)
                for ec in range(EC):
                    es = slice(ec * P, (ec + 1) * P)
                    po = ps.tile([P, L], F32, tag="mm512")
                    for dc in range(DC):
                        nc.tensor.matmul(po[:], wts["wo"][dc][:, es], ao[dc][:, xs],
                                         start=(dc == 0), stop=(dc == DC - 1))
                    if ti == 0:
                        nc.scalar.activation(memo[ec][:], po[:], F.Identity,
                                             bias=consts["ob"][ec][:], scale=consts["os_"][ec][:])
                    else:
                        u = sc.tile([P, L], F32, tag="u")
                        nc.scalar.activation(u[:], po[:], F.Identity,
                                             bias=consts["ob"][ec][:], scale=consts["os_"][ec][:])
                        nc.vector.scalar_tensor_tensor(memo[ec][:], memo[ec][:], 0.5, u[:],
                                                       ALU.mult, ALU.add)
                        nc.gpsimd.tensor_sub(memo[ec][:], memo[ec][:], prev_spk[ec][:])
                    spk = sp8.tile([P, L], BF16, tag="ospk")
                    nc.vector.tensor_scalar(spk[:], memo[ec][:], 1.0, None, ALU.is_ge)
                    prev_spk[ec] = spk
                    nc.sync.dma_start(out_d[es, xs], spk[:])

    nc.compile()
    _PROGRAM_CACHE["nc"] = nc
    return nc


def _rne11(a):
    """Round float32 array to 12-bit significands (RNE) - replicates the PE's
    fp32r operand rounding exactly (verified on hardware)."""
    m, e = np.frexp(np.asarray(a, np.float32).astype(np.float64))
    return np.ldexp(np.rint(m * 4096.0) / 4096.0, e).astype(np.float32)


def _bf16(a):
    return np.asarray(a, np.float32).astype(ml_dtypes.bfloat16)


def kernel(**inputs):
    nc = _build_program()

    f64 = np.float64
    x = np.asarray(inputs["x"], np.float32)

    def bn_fold(g, b_, rm, rv):
        s = (g.astype(f64) / np.sqrt(rv.astype(f64) + EPS))
        bias = b_.astype(f64) - rm.astype(f64) * s
        return s, bias

    sq, bq = bn_fold(inputs["q_g"], inputs["q_b"], inputs["q_rm"], inputs["q_rv"])
    sk, bk = bn_fold(inputs["k_g"], inputs["k_b"], inputs["k_rm"], inputs["k_rv"])
    sv, bv = bn_fold(inputs["v_g"], inputs["v_b"], inputs["v_rm"], inputs["v_rv"])
    so, bo = bn_fold(inputs["o_g"], inputs["o_b"], inputs["o_rm"], inputs["o_rv"])
    C = HD ** -0.5
    # o path: out = bn(lin + o_bias) -> bias' = (o_bias - rm)*s + b
    bo = bo + inputs["o_bias"].astype(f64) * so

    wq = np.ascontiguousarray(inputs["q_w"].astype(f64).T).astype(np.float32)
    wk = np.ascontiguousarray((inputs["k_w"].astype(f64) * (C * sk)[:, None]).T).astype(np.float32)
    wv = np.ascontiguousarray((inputs["v_w"].astype(f64) * sv[:, None]).T).astype(np.float32)
    wo = np.ascontiguousarray(inputs["o_w"].astype(f64).T).astype(np.float32)
    kb_fold = (C * bk).astype(np.float32)
    vb_fold = bv.astype(np.float32)

    shared = {
        "wq": wq, "wk": wk, "wv": wv, "wo": wo,
        "wq8": _bf16(wq),
        "wv8": _bf16(wv),
        "wvl8": _bf16(wv - _rne11(wv)),
        "qs": sq.astype(np.float32).reshape(D, 1),
        "qb": bq.astype(np.float32).reshape(D, 1),
        "os_": so.astype(np.float32).reshape(D, 1),
        "ob": bo.astype(np.float32).reshape(D, 1),
        "kbb": np.ascontiguousarray(np.broadcast_to(kb_fold[None, :], (P, D))),
        "vthr1": np.ascontiguousarray(np.broadcast_to((np.float32(1.0) - vb_fold)[None, :], (P, D))),
        "vthr2": np.ascontiguousarray(np.broadcast_to((np.float32(-1.0) - vb_fold)[None, :], (P, D))),
    }

    in_maps = []
    for b in range(B):
        xT = np.ascontiguousarray(x[b].reshape(NT, D).T)   # (D, NT) f32
        m = dict(shared)
        m["xT"] = xT
        m["xlT"] = _bf16(xT - _rne11(xT))
        m["xbT"] = _bf16(xT)
        in_maps.append(m)

    global _last_in_maps
    _last_in_maps = in_maps
    res = run_bass_kernel_spmd(nc, in_maps, core_ids=list(range(B)))
    outs = []
    for b in range(B):
        oT = res.results[b]["out"]                    # (D, NT) bf16
        outs.append(oT.reshape(D, T, L).transpose(1, 2, 0))
    return np.stack(outs).astype(np.float32)


if __name__ == "__main__":
    import importlib.util
    spec = importlib.util.spec_from_file_location("reference", "/root/problem/reference.py")
    ref = importlib.util.module_from_spec(spec)
    spec.loader.exec_module(ref)
    inp = {k: np.asarray(v) for k, v in ref.setup_inputs().items()}
    exp = np.asarray(ref.reference(**inp))
    act = kernel(**inp)
    rel = np.linalg.norm(act - exp) / np.linalg.norm(exp)
    print("flips:", int(np.sum(act != exp)), "/", exp.size)
    print("Relative error:", rel)


# revision 21
# speedup vs baseline: 1.8136x; 1.3858x over previous
"""Spikformer block (Q/K/V linear+BN+{LIF,ReLU,ternary} -> headwise linear attention
-> O linear+BN+LIF) on 8 TRN2 NeuronCores, data-parallel over batch.

Self-contained: hardcodes shapes; builds one SPMD Bass program; shards batch
across 8 cores; gathers/transposes on host.

Key algebra / precision plan:
  - attention has no softmax -> (q k^T) v reassociated as q (k^T v); per-head
    k^T v is 64x64, ~8x fewer MACs and no 512x512 attn matrix to evict.
  - all BatchNorms are eval-mode affine: folded into per-channel scale/bias on
    host.
  - matmul dtypes tuned per-path to the LIF/ternary flip sensitivity
    (measured on the reference inputs; fp32r rounds operands to 12-bit
    significands (RNE, verified on silicon) and streams at 1 cyc/row for
    free dim >= 256):
      * Q linear: 1 fp32r pass
      * K linear: 1 fp32r pass   (errors average out in the k^T v sum)
      * V linear: 3 passes (fp32r + bf16 x-residual + fp32r w-residual);
        ternary thresholding is the most flip-sensitive spot.
      * O linear: 1 fp32r pass
      * attention: fp16 single-pass (q binary and v ternary are exact in
        fp16; k/kv quantization adds ~1e-4 relative, tolerable).
  - k^T v computed as one 128-wide matmul per token chunk covering 2 heads;
    the cross-head off-diagonal blocks are garbage but never read: only the
    two diagonal 64x64 blocks are copied into pre-zeroed kv tiles.
  - phases interleaved per x-quarter (K,Q,V) and per-timestep (attention/O)
    so DMA and eviction chains overlap PE work.
"""
import sys
for p in ("/opt/trn_rl_repo",):
    if p not in sys.path:
        sys.path.insert(0, p)

import numpy as np
import ml_dtypes

import concourse.bass as bass
import concourse.bacc as bacc
import concourse.mybir as mybir
import concourse.tile as tile
from concourse.bass_utils import run_bass_kernel_spmd

B, T, L, D, H = 8, 4, 512, 512, 8
HD = D // H
NT = T * L            # 2048 tokens per core
P = 128
EC = D // P           # 4 e-chunks
DC = D // P           # 4 d-chunks
TC = NT // P          # 16 token chunks
EPS = 1e-5
F32 = mybir.dt.float32
F32R = mybir.dt.float32r
BF16 = mybir.dt.bfloat16
FP16 = mybir.dt.float16
F = mybir.ActivationFunctionType
ALU = mybir.AluOpType

_PROGRAM_CACHE = {}
_last_in_maps = None


def _build_program():
    if "nc" in _PROGRAM_CACHE:
        return _PROGRAM_CACHE["nc"]

    nc = bacc.Bacc("TRN2", target_bir_lowering=False, debug=False, num_devices=8)

    dram = {}
    # x and weights are packed on host into [128, n*512] layouts so each
    # SBUF tile fills with a single DMA (HWDGE slots are ~630ns each).
    dram["xp"] = nc.dram_tensor("xp", [P, T * DC * L], F32R, kind="ExternalInput")
    dram["xlp"] = nc.dram_tensor("xlp", [P, T * DC * L], BF16, kind="ExternalInput")
    for w in ("wq", "wk", "wv", "wvl32", "wo"):
        dram[w] = nc.dram_tensor(w, [P, DC * D], F32R, kind="ExternalInput")
    dram["wv8"] = nc.dram_tensor("wv8", [P, DC * D], BF16, kind="ExternalInput")
    dram["cpack"] = nc.dram_tensor("cpack", [P, 16], F32, kind="ExternalInput")
    dram["obrow"] = nc.dram_tensor("obrow", [1, 2 * D + L], F32R, kind="ExternalInput")
    dram["kbb"] = nc.dram_tensor("kbb", [P, D], F32, kind="ExternalInput")
    dram["vthrp"] = nc.dram_tensor("vthrp", [P, 2 * D], F32, kind="ExternalInput")
    out_d = nc.dram_tensor("out", [D, NT], BF16, kind="ExternalOutput")

    with tile.TileContext(nc) as tc_:
        with tc_.tile_pool(name="sb", bufs=1) as sb, \
             tc_.tile_pool(name="sc", bufs=3) as sc, \
             tc_.tile_pool(name="sp8", bufs=6) as sp8, \
             tc_.tile_pool(name="ps", bufs=5, space="PSUM") as ps, \
             tc_.tile_pool(name="pk", bufs=3, space="PSUM") as pk:

            # ---------- persistent SBUF tiles ----------
            # x per quarter, dc-major columns: xq[q4][:, dc*L + t]
            xq = [sb.tile([P, DC * L], F32R, tag="xa", bufs=DC, name=f"xq{i}") for i in range(T)]
            xlq = [sb.tile([P, DC * L], BF16, tag="xl", bufs=DC, name=f"xlq{i}") for i in range(T)]
            wts = {}
            for w in ("wq", "wk", "wv", "wvl32", "wo"):
                wts[w] = sb.tile([P, DC * D], F32R, tag=w, name=w)
            wts["wv8"] = sb.tile([P, DC * D], BF16, tag="wv8", name="wv8")
            qT16 = [sb.tile([P, NT], FP16, tag="qT", bufs=EC, name=f"qT{i}") for i in range(EC)]
            k16 = [sb.tile([P, D], FP16, tag="k16", bufs=TC, name=f"k16_{i}") for i in range(TC)]
            v16 = [sb.tile([P, D], FP16, tag="v16", bufs=TC, name=f"v16_{i}") for i in range(TC)]
            kv16 = [sb.tile([P, P], FP16, tag="kv16", bufs=TC, name=f"kv16_{i}") for i in range(TC)]
            # attention output [e, t] fp32r; reuses the xT buffers (tag "xa"),
            # safe because all xT reads precede phase 2.
            ao = [sb.tile([P, NT], F32R, tag="xa", bufs=DC, name=f"ao{i}") for i in range(DC)]
            memq = [sb.tile([P, L], F32, tag="memq", bufs=EC, name=f"memq{i}") for i in range(EC)]
            obrow = sb.tile([1, 2 * D + L], F32R, tag="obrow")
            onesrow = obrow[:, 2 * D:2 * D + L]
            memo = [sb.tile([P, L], F32, tag="memo", bufs=EC, name=f"memo{i}") for i in range(EC)]
            cpack = sb.tile([P, 16], F32, tag="cst")
            consts = {v_: [cpack[:, (vi * 4 + i):(vi * 4 + i + 1)] for i in range(EC)]
                      for vi, v_ in enumerate(("qs", "qb", "os_", "ob"))}
            kbbt = sb.tile([P, D], F32, tag="kbb")
            kbb = kbbt[:]
            vthrp = sb.tile([P, 2 * D], F32, tag="thr")
            vthr1 = vthrp[:, 0:D]
            vthr2 = vthrp[:, D:2 * D]

            # kv tiles: zero once; only diagonal 64x64 blocks are ever written
            for i in range(TC):
                nc.gpsimd.memset(kv16[i][:], 0.0)

            # ---------- loads ----------
            # single queue (SP), exact consumption order; DMAs on compute
            # queues would steal their sequencers (~667ns per issue).
            W = DC * L
            nc.sync.dma_start(wts["wk"][:, 0:D], dram["wk"][:, 0:D])

            def ldx(q4):
                for hh in range(2):
                    hs = slice(hh * W // 2, (hh + 1) * W // 2)
                    nc.sync.dma_start(xq[q4][:, hs],
                                      dram["xp"][:, q4 * W + hh * W // 2:q4 * W + (hh + 1) * W // 2])

            nc.sync.dma_start(xq[0][:, 0:W // 2], dram["xp"][:, 0:W // 2])
            nc.sync.dma_start(wts["wk"][:, D:DC * D], dram["wk"][:, D:DC * D])
            nc.sync.dma_start(xq[0][:, W // 2:W], dram["xp"][:, W // 2:W])
            nc.sync.dma_start(kbbt[:], dram["kbb"][:])
            ldx(1)
            ldx(2)
            ldx(3)
            nc.sync.dma_start(wts["wq"][:], dram["wq"][:])
            nc.sync.dma_start(cpack[:], dram["cpack"][:])
            for q4 in range(T):
                nc.sync.dma_start(xlq[q4][:], dram["xlp"][:, q4 * W:(q4 + 1) * W])
            nc.sync.dma_start(vthrp[:], dram["vthrp"][:])
            nc.sync.dma_start(wts["wv"][:], dram["wv"][:])
            nc.sync.dma_start(wts["wvl32"][:], dram["wvl32"][:])
            nc.sync.dma_start(wts["wv8"][:], dram["wv8"][:])
            nc.sync.dma_start(wts["wo"][:], dram["wo"][:])
            nc.sync.dma_start(obrow[:], dram["obrow"][:])

            # ---------- phase 1: per x-quarter: K tiles, Q tile, then V of
            # the PREVIOUS quarter (gives the DMA a quarter of slack to land
            # the V weights and xlT residuals) ----
            def emit_v(q4):
                for mc in range(4):
                    tc2 = q4 * 4 + mc
                    mc4 = mc
                    pvv = ps.tile([P, D], F32, tag="mm512", name=f"pv{tc2}")
                    for dc in range(DC):
                        nc.tensor.matmul(pvv[:], xq[q4][:, dc * L + mc4 * P:dc * L + (mc4 + 1) * P],
                                         wts["wv"][:, dc * D:(dc + 1) * D],
                                         start=(dc == 0), stop=False)
                    for dc in range(DC):
                        nc.tensor.matmul(pvv[:], xq[q4][:, dc * L + mc4 * P:dc * L + (mc4 + 1) * P],
                                         wts["wvl32"][:, dc * D:(dc + 1) * D],
                                         start=False, stop=False)
                    for dc in range(DC):
                        nc.tensor.matmul(pvv[:], xlq[q4][:, dc * L + mc4 * P:dc * L + (mc4 + 1) * P],
                                         wts["wv8"][:, dc * D:(dc + 1) * D],
                                         start=False, stop=(dc == DC - 1))
                    t1 = sc.tile([P, D], FP16, tag="t1", name=f"t1_{tc2}")
                    t2 = sc.tile([P, D], FP16, tag="t2", name=f"t2_{tc2}")
                    nc.vector.tensor_tensor(t1[:], pvv[:], vthr1, ALU.is_ge)
                    nc.vector.tensor_tensor(t2[:], pvv[:], vthr2, ALU.is_le)
                    nc.gpsimd.tensor_sub(v16[tc2][:], t1[:], t2[:])

            # all K tiles first (needs only wk + xT), then Q (wq), then V
            for tc2 in range(TC):
                q4k, mc4 = tc2 // 4, tc2 % 4
                pkv = ps.tile([P, D], F32, tag="mm512", name=f"pk{tc2}")
                for dc in range(DC):
                    nc.tensor.matmul(pkv[:], xq[q4k][:, dc * L + mc4 * P:dc * L + (mc4 + 1) * P],
                                     wts["wk"][:, dc * D:(dc + 1) * D],
                                     start=(dc == 0), stop=(dc == DC - 1))
                kf = sc.tile([P, D], F32, tag="kf", name=f"kf{tc2}")
                nc.vector.tensor_tensor(kf[:], pkv[:], kbb, ALU.add)
                nc.scalar.activation(k16[tc2][:], kf[:], F.Relu)

            for ti in range(T):
                xs = slice(ti * L, (ti + 1) * L)
                for ec in range(EC):
                    es = slice(ec * P, (ec + 1) * P)
                    pq = ps.tile([P, L], F32, tag="mm512", name=f"pq{ti}_{ec}")
                    for dc in range(DC):
                        nc.tensor.matmul(pq[:], wts["wq"][:, dc * D + ec * P:dc * D + (ec + 1) * P],
                                         xq[ti][:, dc * L:(dc + 1) * L],
                                         start=(dc == 0), stop=(dc == DC - 1))
                    if ti == 0:
                        nc.scalar.activation(memq[ec][:], pq[:], F.Identity,
                                             bias=consts["qb"][ec], scale=consts["qs"][ec])
                    else:
                        u = sc.tile([P, L], F32, tag="u", name=f"u{ti}_{ec}")
                        nc.scalar.activation(u[:], pq[:], F.Identity,
                                             bias=consts["qb"][ec], scale=consts["qs"][ec])
                        nc.vector.scalar_tensor_tensor(memq[ec][:], memq[ec][:], 0.5, u[:],
                                                       ALU.mult, ALU.add)
                        nc.gpsimd.tensor_sub(memq[ec][:], memq[ec][:],
                                             qT16[ec][:, (ti - 1) * L:ti * L])
                    nc.gpsimd.tensor_scalar(qT16[ec][:, xs], memq[ec][:], 1.0, None, ALU.is_ge)
            for q4 in range(T):
                emit_v(q4)

            # ---------- phase 2+3: attention and O-linear, interleaved per ti
            prev_spk = [None] * EC

            def emit_o(ti, ecs=range(EC)):
                # os is folded into wo on host, so psum holds os*lin; bias ob
                # is added by act (mid steps) or a K=1 matmul (final step).
                xs = slice(ti * L, (ti + 1) * L)
                final = (ti == T - 1)
                for ec in ecs:
                    es = slice(ec * P, (ec + 1) * P)
                    po = ps.tile([P, L], F32, tag="mm512")
                    for dc in range(DC):
                        nc.tensor.matmul(po[:], wts["wo"][:, dc * D + ec * P:dc * D + (ec + 1) * P],
                                         ao[dc][:, xs],
                                         start=(dc == 0), stop=(dc == DC - 1 and not final))
                    if final:
                        nc.tensor.matmul(po[:], obrow[:, es], onesrow,
                                         start=False, stop=False)
                        nc.tensor.matmul(po[:], obrow[:, D + ec * P:D + (ec + 1) * P], onesrow,
                                         start=False, stop=True)
                    spk = sp8.tile([P, L], BF16, tag="ospk")
                    if ti == 0:
                        nc.scalar.activation(memo[ec][:], po[:], F.Identity,
                                             bias=consts["ob"][ec])
                        nc.gpsimd.tensor_scalar(spk[:], memo[ec][:], 1.0, None, ALU.is_ge)
                    elif not final:
                        u = sc.tile([P, L], F32, tag="u")
                        nc.scalar.activation(u[:], po[:], F.Identity,
                                             bias=consts["ob"][ec])
                        nc.vector.scalar_tensor_tensor(memo[ec][:], memo[ec][:], 0.5, u[:],
                                                       ALU.mult, ALU.add)
                        nc.gpsimd.tensor_sub(memo[ec][:], memo[ec][:], prev_spk[ec][:])
                        nc.gpsimd.tensor_scalar(spk[:], memo[ec][:], 1.0, None, ALU.is_ge)
                    else:
                        # final step: bias already in psum; act stage skipped;
                        # halves pipeline the tail chain
                        for hh in range(2):
                            h = slice(hh * L // 2, (hh + 1) * L // 2)
                            hx = slice(ti * L + hh * L // 2, ti * L + (hh + 1) * L // 2)
                            nc.vector.scalar_tensor_tensor(memo[ec][:, h], memo[ec][:, h],
                                                           0.5, po[:, h], ALU.mult, ALU.add)
                            eng = nc.gpsimd if (ec + hh) % 2 == 0 else nc.vector
                            eng.tensor_sub(memo[ec][:, h], memo[ec][:, h], prev_spk[ec][:, h])
                            eng.tensor_scalar(spk[:, h], memo[ec][:, h], 1.0, None, ALU.is_ge)
                            nc.sync.dma_start(out_d[es, hx], spk[:, h])
                        prev_spk[ec] = spk
                        continue
                    prev_spk[ec] = spk
                    nc.sync.dma_start(out_d[es, xs], spk[:])

            for ti in range(T):
                xs = slice(ti * L, (ti + 1) * L)
                pkvs = []
                for c in range(EC):          # head pair (2c, 2c+1) == e-chunk c
                    es = slice(c * P, (c + 1) * P)
                    pkv64 = pk.tile([P, P], F32, tag="kv64", name=f"pkv{ti}_{c}")
                    for mc in range(4):
                        tc2 = ti * 4 + mc
                        nc.tensor.matmul(pkv64[:], k16[tc2][:, es], v16[tc2][:, es],
                                         start=(mc == 0), stop=(mc == 3))
                    pkvs.append(pkv64)
                for c in range(EC):
                    kvt = kv16[ti * 4 + c]
                    nc.scalar.copy(kvt[0:HD, 0:HD], pkvs[c][0:HD, 0:HD])
                    nc.scalar.copy(kvt[HD:P, HD:P], pkvs[c][HD:P, HD:P])
                # O-linear of the previous timestep fills PE while the act
                # engine drains the kv copies (first half) and the ao copies
                # (second half).
                if ti > 0:
                    emit_o(ti - 1, (0, 1))
                for c in range(EC):
                    pso = ps.tile([P, L], F32, tag="mm512", name=f"pso{ti}_{c}")
                    nc.tensor.matmul(pso[:], kv16[ti * 4 + c][:], qT16[c][:, xs],
                                     start=True, stop=True)
                    nc.vector.tensor_copy(ao[c][:, xs], pso[:])
                if ti > 0:
                    emit_o(ti - 1, (2, 3))
            emit_o(T - 1)

    nc.compile()
    _PROGRAM_CACHE["nc"] = nc
    return nc


def _rne11(a):
    """Round float32 array to 12-bit significands (RNE) - replicates the PE's
    fp32r operand rounding exactly (verified on hardware)."""
    m, e = np.frexp(np.asarray(a, np.float32).astype(np.float64))
    return np.ldexp(np.rint(m * 4096.0) / 4096.0, e).astype(np.float32)


def _bf16(a):
    return np.asarray(a, np.float32).astype(ml_dtypes.bfloat16)


def kernel(**inputs):
    nc = _build_program()

    f64 = np.float64
    x = np.asarray(inputs["x"], np.float32)

    def bn_fold(g, b_, rm, rv):
        s = (g.astype(f64) / np.sqrt(rv.astype(f64) + EPS))
        bias = b_.astype(f64) - rm.astype(f64) * s
        return s, bias

    sq, bq = bn_fold(inputs["q_g"], inputs["q_b"], inputs["q_rm"], inputs["q_rv"])
    sk, bk = bn_fold(inputs["k_g"], inputs["k_b"], inputs["k_rm"], inputs["k_rv"])
    sv, bv = bn_fold(inputs["v_g"], inputs["v_b"], inputs["v_rm"], inputs["v_rv"])
    so, bo = bn_fold(inputs["o_g"], inputs["o_b"], inputs["o_rm"], inputs["o_rv"])
    C = HD ** -0.5
    # o path: out = bn(lin + o_bias) -> bias' = (o_bias - rm)*s + b
    bo = bo + inputs["o_bias"].astype(f64) * so

    wq = np.ascontiguousarray(inputs["q_w"].astype(f64).T).astype(np.float32)
    wk = np.ascontiguousarray((inputs["k_w"].astype(f64) * (C * sk)[:, None]).T).astype(np.float32)
    wv = np.ascontiguousarray((inputs["v_w"].astype(f64) * sv[:, None]).T).astype(np.float32)
    wo = np.ascontiguousarray(inputs["o_w"].astype(f64).T).astype(np.float32)
    kb_fold = (C * bk).astype(np.float32)
    vb_fold = bv.astype(np.float32)

    def wpack(w):
        return np.ascontiguousarray(np.concatenate(
            [w[dc * P:(dc + 1) * P, :] for dc in range(DC)], axis=1))

    wvl32 = wv - _rne11(wv)
    shared = {
        "wq": wpack(wq), "wk": wpack(wk), "wv": wpack(wv),
        "wo": wpack(wo * so.astype(np.float32)[None, :]),
        "obrow": np.concatenate([_rne11(bo.astype(np.float32)),
                                 bo.astype(np.float32) - _rne11(bo.astype(np.float32)),
                                 np.ones(L, np.float32)]).reshape(1, 2 * D + L),
        "wvl32": wpack(wvl32),
        "wv8": _bf16(wpack(wv)),
        "cpack": np.stack([v.astype(np.float32).reshape(EC, P).T.reshape(P, EC)
                           for v in (sq, bq, so, bo)], axis=1).reshape(P, 16),
        "kbb": np.ascontiguousarray(np.broadcast_to(kb_fold[None, :], (P, D))),
        "vthrp": np.ascontiguousarray(np.concatenate([
            np.broadcast_to((np.float32(1.0) - vb_fold)[None, :], (P, D)),
            np.broadcast_to((np.float32(-1.0) - vb_fold)[None, :], (P, D)),
        ], axis=1)),
    }

    in_maps = []
    for b in range(B):
        xT = x[b].reshape(NT, D).T                         # (D, NT) f32
        xl = _bf16(xT - _rne11(xT))
        # pack quarter-major, dc-major: xp[:, (q4*DC + dc)*L + t] = xT[dc-chunk, q4-quarter]
        xp = np.concatenate([xT[dc * P:(dc + 1) * P, q4 * L:(q4 + 1) * L]
                             for q4 in range(T) for dc in range(DC)], axis=1)
        xlp = np.concatenate([xl[dc * P:(dc + 1) * P, q4 * L:(q4 + 1) * L]
                              for q4 in range(T) for dc in range(DC)], axis=1)
        m = dict(shared)
        m["xp"] = np.ascontiguousarray(xp)
        m["xlp"] = np.ascontiguousarray(xlp)
        in_maps.append(m)

    global _last_in_maps
    _last_in_maps = in_maps
    res = run_bass_kernel_spmd(nc, in_maps, core_ids=list(range(B)))
    outs = []
    for b in range(B):
        oT = res.results[b]["out"]                    # (D, NT) bf16
        outs.append(oT.reshape(D, T, L).transpose(1, 2, 0))
    return np.stack(outs).astype(np.float32)


if __name__ == "__main__":
    import importlib.util
    spec = importlib.util.spec_from_file_location("reference", "/root/problem/reference.py")
    ref = importlib.util.module_from_spec(spec)
    spec.loader.exec_module(ref)
    inp = {k: np.asarray(v) for k, v in ref.setup_inputs().items()}
    exp = np.asarray(ref.reference(**inp))
    act = kernel(**inp)
    rel = np.linalg.norm(act - exp) / np.linalg.norm(exp)
    print("flips:", int(np.sum(act != exp)), "/", exp.size)
    print("Relative error:", rel)
